# revision 15
# baseline (speedup 1.0000x reference)
"""KNN classifier kernel for Trainium2 (8 NeuronCores, Bass/Tile).

Problem (nn_KNNClassifier): given queries x [4096, 512], train bank
x_train [65536, 512], labels y_train [65536] (100 classes), compute for
each query the top-200 neighbors by dot-product similarity, weight them
by exp(sim/0.1), accumulate per-class scores, and return the descending
argsort of class scores -> int32 [4096, 100].

Key structural fact: with T=0.1 every top-200 sim (~70+) overflows
exp(sim/T) to +inf in fp32, so the reference ranking collapses to
"classes present among the top-200 (ascending) then absent (ascending)".
The device therefore only needs to FLAG, per (query, class-slot), whether
the slot could contain a top-200 member; the host recomputes flagged
slots exactly and replays the reference-equivalent accumulation.

Device strategy (shard train bank over N across 8 cores):
  - Host reorders x_train columns by class into a shared 4x2048-column
    grid per core (same slot structure on all cores; only the class
    assigned to each slot differs). Each slot occupies the same offset
    range in both halves of its group, so a single tensor-tensor max
    folds same-class column pairs.
  - Per core, per (group, query-block): 8 fp8 DoubleRow matmuls
    (contraction 256 each) -> PSUM [128, 2048] f32; the scalar engine
    casts the h1 half [0:1024] to SBUF bf16 (banks 1-2) while one raw
    InstTensorTensor max on the vector engine folds that bf16 tile
    against the h2 half read directly from PSUM f32 (banks 3-4) --
    one op per engine per iteration, so neither engine chain ever
    backpressures the PE's PSUM rotation. The folded tile is DMA'd to
    HBM.
  - Host decodes the folded bf16 sims, estimates the per-query top-200
    threshold, flags slots whose folded max is near/above it, recomputes
    those slots exactly in fp32, and runs the exact accumulation
    (fp32 exp -> scatter-add -> stable argsort), with per-query full
    fallback for ties/pathologies.
"""

import os
import sys

for _p in ("/opt/trn_rl_repo",):
    if _p not in sys.path and os.path.isdir(_p):
        sys.path.insert(0, _p)

import numpy as np

import concourse.mybir as mybir
import concourse.tile as tile
from concourse import bacc
from concourse.bass_utils import run_bass_kernel_spmd

# Problem shapes (hardcoded per spec)
B, N, D = 4096, 65536, 512
NUM_CLASSES = 100
KNN_K = 200
KNN_T = 0.1
NCORES = 8

KT = D // 128  # 4 contraction tiles
QB = B // 128  # 32 query blocks of 128
NG = 4  # PSUM groups per core
GW = 2048  # columns per group (4 PSUM banks)
HALF = GW // 2
CPC = NG * GW  # 8192 columns per core
FPC = NG * HALF  # 4096 folded columns per core
WARM_MMS = 24  # dummy matmuls issued at t=0 so HAM un-throttles the PE
# (and the DMA-wait head overlaps the 3.4us warmup window)
XCH = 8  # x DMA chunks (512 queries each)

SLACK = 5.6  # fp8 matmul noise (~4.5 sigma) + bf16 rounding
CUT = SLACK + 3.2  # slot-flag margin below the t0 threshold estimate
NEG = -1.0e30

_PROG = None
LAST_INFO = {}


def _tt_max(nc, out_ap, a_ap, b_ap):
    """Raw elementwise tensor-tensor max on the vector engine.

    bass has no public wrapper for InstTensorTensor, but it is the only
    DVE op with a 2x_1p uop (2 results/cycle on packed bf16) — the
    scalar_tensor_tensor wrapper runs at 1x.
    """
    ve = nc.vector
    return ve.add_instruction(
        mybir.InstTensorTensor(
            name=nc.get_next_instruction_name(),
            op=mybir.AluOpType.max,
            ins=[ve.lower_ap(a_ap), ve.lower_ap(b_ap)],
            outs=[ve.lower_ap(out_ap)],
        )
    )


def _build_program():
    nc = bacc.Bacc(
        "TRN2", target_bir_lowering=False, debug=False, num_devices=NCORES
    )
    f32 = mybir.dt.float32
    bf16 = mybir.dt.bfloat16
    f8 = mybir.dt.float8e4
    XW = B // XCH

    xT_d = nc.dram_tensor("xT", (D, B), f8, kind="ExternalInput").ap()
    wT_d = nc.dram_tensor("wT", (D, CPC), f8, kind="ExternalInput").ap()
    fold_d = nc.dram_tensor("fold", (B, FPC), bf16, kind="ExternalOutput").ap()

    from contextlib import ExitStack

    with tile.TileContext(nc) as tc:
        # PE warmup: dummy matmuls on garbage data, issued before any DMA
        # dependency, so the HAM clock-gate reaches 8/8 while the startup
        # DMAs are still in flight. Scoped pools so the PSUM bank is
        # returned before the main loop's double-buffered tiles claim
        # all 8 banks.
        with tc.tile_pool(name="warms", bufs=1) as wsp, tc.tile_pool(
            name="warmp", bufs=1, space="PSUM"
        ) as wpp:
            junk = wsp.tile([128, 2, 640], f8, tag="junk")
            nc.vector.memset(junk[:], 1.0)
            wps = wpp.tile([128, 512], f32, tag="wps")
            for r in range(WARM_MMS):
                nc.tensor.matmul(
                    wps[:],
                    junk[:, :, 0:128],
                    junk[:, :, 128:640],
                    start=True,
                    stop=True,
                    perf_mode=mybir.MatmulPerfMode.DoubleRow,
                )

        with ExitStack() as ctx:
            xpool = ctx.enter_context(tc.tile_pool(name="xp", bufs=1))
            wpool = ctx.enter_context(tc.tile_pool(name="wp", bufs=2))
            spool = ctx.enter_context(tc.tile_pool(name="sp", bufs=3))
            fpool = ctx.enter_context(tc.tile_pool(name="fp", bufs=3))
            ppool = ctx.enter_context(tc.tile_pool(name="pp", bufs=2, space="PSUM"))

            xsb = xpool.tile([128, KT, B], f8, tag="x")

            for g in range(NG):
                wt = wpool.tile([128, KT, GW], f8, tag="w")
                if g == 0:
                    # First-use-ordered startup: the first matmul
                    # (kp=0, t=0, block 0) needs x k0/k1 of query chunk 0
                    # plus w k0/k1 cols [0:512] -- ~256 KB -- then the
                    # rest streams in first-use order.
                    for kp in range(KT // 2):
                        for k in (2 * kp, 2 * kp + 1):
                            nc.sync.dma_start(
                                xsb[:, k, 0:XW],
                                xT_d[k * 128 : (k + 1) * 128, 0:XW],
                            )
                        for t in range(GW // 512):
                            for k in (2 * kp, 2 * kp + 1):
                                nc.sync.dma_start(
                                    wt[:, k, t * 512 : (t + 1) * 512],
                                    wT_d[
                                        k * 128 : (k + 1) * 128,
                                        t * 512 : (t + 1) * 512,
                                    ],
                                )
                    for c in range(1, XCH):
                        for k in range(KT):
                            nc.sync.dma_start(
                                xsb[:, k, c * XW : (c + 1) * XW],
                                xT_d[k * 128 : (k + 1) * 128, c * XW : (c + 1) * XW],
                            )
                else:
                    for k in range(KT):
                        nc.sync.dma_start(
                            wt[:, k, :],
                            wT_d[k * 128 : (k + 1) * 128, g * GW : (g + 1) * GW],
                        )
                for b in range(QB):
                    ps = ppool.tile([128, GW], f32, tag="ps")
                    for kp in range(KT // 2):
                        for t in range(GW // 512):
                            nc.tensor.matmul(
                                ps[:, t * 512 : (t + 1) * 512],
                                xsb[:, 2 * kp : 2 * kp + 2, b * 128 : (b + 1) * 128],
                                wt[:, 2 * kp : 2 * kp + 2, t * 512 : (t + 1) * 512],
                                start=(kp == 0),
                                stop=(kp == KT // 2 - 1),
                                perf_mode=mybir.MatmulPerfMode.DoubleRow,
                            )
                    sbf = spool.tile([128, HALF], bf16, tag="s")
                    nc.scalar.copy(sbf[:], ps[:, 0:HALF])
                    f1 = fpool.tile([128, HALF], bf16, tag="f")
                    _tt_max(nc, f1[:], sbf[:], ps[:, HALF:GW])
                    nc.sync.dma_start(
                        fold_d[
                            b * 128 : (b + 1) * 128,
                            g * HALF : (g + 1) * HALF,
                        ],
                        f1[:],
                    )

    nc.compile()
    return nc


def _get_program():
    global _PROG
    if _PROG is None:
        _PROG = _build_program()
    return _PROG


def _plan_layout(y_train):
    """Class-pure slot layout on a shared 4x2048 grid, identical across cores.

    Each slot j has a fixed half-width h[j] and (group, offset) shared by
    all cores; core i places its j-th largest class there, split into two
    h[j]-column halves at the same offset in each half of the group (so
    the device fold pairs same-class columns). Rows that do not fit are
    computed exactly on the host.

    Returns (colmap, host_rows):
      colmap: int64 [NCORES, CPC] -> original x_train row, -1 pad
      host_rows: int64 [H] train rows computed exactly on the host
    """
    cnt = np.bincount(y_train, minlength=NUM_CLASSES)
    by_class = np.argsort(y_train, kind="stable")
    starts = np.zeros(NUM_CLASSES + 1, dtype=np.int64)
    np.cumsum(cnt, out=starts[1:])

    cls_desc = np.argsort(-cnt, kind="stable")
    core_classes = [[] for _ in range(NCORES)]
    for idx, c in enumerate(cls_desc):
        r, i = divmod(idx, NCORES)
        if r % 2:
            i = NCORES - 1 - i
        core_classes[i].append(int(c))
    S = max(len(cc) for cc in core_classes)
    for cc in core_classes:
        while len(cc) < S:
            cc.append(-1)

    # Shared half-widths: the widest class assigned to each slot position.
    h = np.zeros(S, dtype=np.int64)
    for j in range(S):
        h[j] = max(
            (cnt[cc[j]] + 1) // 2 if cc[j] >= 0 else 0 for cc in core_classes
        )
        h[j] = max(h[j], 1)
    # Fit the grid: total half-capacity is NG * HALF.
    cap = NG * HALF
    while h.sum() > cap:  # shed excess one column at a time, widest first
        j = int(np.argmax(h))
        h[j] -= 1

    # Pack slots into the 4 group bins (first-fit decreasing).
    order = sorted(range(S), key=lambda j: -h[j])
    bin_used = [0] * NG
    slot_group = np.zeros(S, dtype=np.int64)
    slot_off = np.zeros(S, dtype=np.int64)
    for j in order:
        g = min(
            (i for i in range(NG) if bin_used[i] + h[j] <= HALF),
            key=lambda i: HALF - bin_used[i],
            default=None,
        )
        if g is None:
            # Shrink to the largest remaining bin.
            g = int(np.argmax([HALF - u for u in bin_used]))
            h[j] = HALF - bin_used[g]
        slot_group[j] = g
        slot_off[j] = bin_used[g]
        bin_used[g] += h[j]

    colmap = np.full((NCORES, CPC), -1, dtype=np.int64)
    host_rows = []
    for j in range(S):
        g, off, hj = int(slot_group[j]), int(slot_off[j]), int(h[j])
        for i in range(NCORES):
            c = core_classes[i][j]
            if c < 0:
                continue
            n = int(cnt[c])
            rows = by_class[starts[c] : starts[c] + n]
            keep = min(n, 2 * hj)
            n1 = min(hj, keep)
            c1 = g * GW + off
            c2 = g * GW + HALF + off
            colmap[i, c1 : c1 + n1] = rows[0:n1]
            if keep > n1:
                colmap[i, c2 : c2 + keep - n1] = rows[n1:keep]
            if n > keep:
                host_rows.extend(rows[keep:n])

    host_rows = np.array(sorted(host_rows), dtype=np.int64)
    return colmap, host_rows


def _host_merge(x, x_train, y_train, folded, colmap, host_rows):
    """Exact top-200 presence -> class scores -> ranking.

    folded: f32 [NCORES, B, FPC] pair-maxes of fp8 sims (bf16 rounded).
    Values serve only to locate candidates: every folded PAIR that could
    contain a top-200 member gets both of its columns recomputed exactly
    (~300 pairs/query), so no device value is ever used as a final
    candidate. Host FLOPs stay ~3 GFLOP (single-CPU container).
    """
    H = host_rows.shape[0]
    if H:
        hostV = (x @ x_train[host_rows].T).astype(np.float64)  # [B, H]
        host_class = y_train[host_rows]
    else:
        hostV = np.zeros((B, 0))
        host_class = np.zeros(0, dtype=y_train.dtype)

    # t0: estimate of the top-200 threshold from folded values + hostV.
    flat = np.ascontiguousarray(folded.transpose(1, 0, 2)).reshape(
        B, NCORES * FPC
    )
    A0 = np.concatenate([flat, hostV.astype(np.float32)], axis=1)
    kth = A0.shape[1] - KNN_K
    t0 = np.partition(A0, kth, axis=1)[:, kth].astype(np.float64)
    del A0

    # Candidate pairs: folded value near/above t0.
    mask = flat >= (t0[:, None].astype(np.float32) - CUT)
    qidx, fcol = np.nonzero(mask)
    del mask
    LAST_INFO["cand_pairs"] = int(qidx.size)
    core = fcol // FPC
    off = fcol % FPC
    g = off // HALF
    o = off % HALF
    c1 = g * GW + o
    c2 = g * GW + HALF + o
    r1 = colmap[core, c1]
    r2 = colmap[core, c2]
    rows2 = np.stack([r1, r2], axis=1).ravel()
    q2 = np.repeat(qidx, 2)
    valid = rows2 >= 0
    rows2 = rows2[valid]
    q2 = q2[valid]

    # Exact sims for candidate columns (chunked row-wise dot products).
    vals = np.empty(rows2.size, dtype=np.float64)
    CH = 65536
    for s in range(0, rows2.size, CH):
        e = min(s + CH, rows2.size)
        vals[s:e] = np.einsum(
            "nd,nd->n", x_train[rows2[s:e]], x[q2[s:e]], optimize=True
        )
    labels2 = y_train[rows2]

    # Per-query candidate table [B, MAXC] (+ hostV appended).
    counts = np.bincount(q2, minlength=B)
    MAXC = int(counts.max())
    starts = np.zeros(B + 1, dtype=np.int64)
    np.cumsum(counts, out=starts[1:])
    # q2 is already sorted (nonzero row-major); position within query:
    pos = np.arange(q2.size) - starts[q2]
    Acand = np.full((B, MAXC), NEG, dtype=np.float64)
    Lcand = np.zeros((B, MAXC), dtype=np.int64)
    Acand[q2, pos] = vals
    Lcand[q2, pos] = labels2

    A = np.concatenate([Acand, hostV], axis=1)
    L = np.concatenate(
        [Lcand, np.broadcast_to(host_class, (B, H))], axis=1
    )
    kth = A.shape[1] - KNN_K
    t1 = np.partition(A, kth, axis=1)[:, kth]
    sel = np.argpartition(-A, KNN_K - 1, axis=1)[:, :KNN_K]
    rowix = np.arange(B)[:, None]
    sel_v = A[rowix, sel]

    full_fallback = set()
    # Boundary ties -> per-query fallback (argpartition splits arbitrarily)
    vmin = sel_v.min(axis=1)
    tie = (A == vmin[:, None]).sum(axis=1) != (sel_v == vmin[:, None]).sum(axis=1)
    for q in np.nonzero(tie)[0]:
        full_fallback.add(int(q))
    # Guards: flag logic assumes t1 close to t0, and positive thresholds.
    for q in np.nonzero((t1 < 1.0) | (t1 < t0 - CUT + SLACK + 0.5))[0]:
        full_fallback.add(int(q))
    LAST_INFO["fallback_rows"] = len(full_fallback)

    labels = L[rowix, sel]  # [B, K]

    scores = np.zeros((B, NUM_CLASSES), dtype=np.float32)
    with np.errstate(over="ignore"):
        wts = np.exp(sel_v.astype(np.float32) / np.float32(KNN_T))
    ok = np.ones(B, dtype=bool)
    for q in full_fallback:
        ok[q] = False
    qs = np.nonzero(ok)[0]
    np.add.at(
        scores,
        (np.repeat(qs, KNN_K), labels[qs].ravel()),
        wts[qs].ravel(),
    )

    if full_fallback:
        qfb = np.array(sorted(full_fallback))
        sims_fb = x[qfb].astype(np.float64) @ x_train.T.astype(np.float64)
        for i, q in enumerate(qfb):
            sims = sims_fb[i]
            cand = np.argpartition(-sims, KNN_K + 56)[: KNN_K + 56]
            order = cand[np.lexsort((cand, -sims[cand]))][:KNN_K]
            lab = y_train[order]
            with np.errstate(over="ignore"):
                wq = np.exp(sims[order].astype(np.float32) / np.float32(KNN_T))
            scores[q] = 0.0
            np.add.at(scores[q], lab, wq)

    return np.argsort(-scores, axis=1, kind="stable").astype(np.int32)


def kernel(x, x_train, y_train):
    x = np.asarray(x, dtype=np.float32)
    x_train = np.asarray(x_train, dtype=np.float32)
    y_train = np.asarray(y_train).astype(np.int64)

    colmap, host_rows = _plan_layout(y_train)
    nc = _get_program()

    f8np = mybir.dt.np(mybir.dt.float8e4)
    xtr_T = np.ascontiguousarray(x_train.T)  # [D, N] f32
    xT = np.ascontiguousarray(x.T).astype(f8np)

    in_maps = []
    for i in range(NCORES):
        wTi = np.zeros((D, CPC), dtype=f8np)
        real = colmap[i] >= 0
        wTi[:, real] = xtr_T[:, colmap[i, real]].astype(f8np)
        in_maps.append({"xT": xT, "wT": wTi})

    res = run_bass_kernel_spmd(nc, in_maps, core_ids=list(range(NCORES)))
    LAST_INFO["exec_time_ns"] = res.exec_time_ns
    LAST_INFO["results"] = res

    folded = np.stack(
        [
            np.asarray(res.results[c]["fold"]).astype(np.float32)
            for c in range(NCORES)
        ]
    )  # [NCORES, B, FPC] f32
    return _host_merge(x, x_train, y_train, folded, colmap, host_rows)


# revision 16
# speedup vs baseline: 1.0879x; 1.0879x over previous
"""KNN classifier kernel for Trainium2 (8 NeuronCores, Bass/Tile).

Problem (nn_KNNClassifier): given queries x [4096, 512], train bank
x_train [65536, 512], labels y_train [65536] (100 classes), compute for
each query the top-200 neighbors by dot-product similarity, weight them
by exp(sim/0.1), accumulate per-class scores, and return the descending
argsort of class scores -> int32 [4096, 100].

Key structural fact: with T=0.1 every top-200 sim (~70+) overflows
exp(sim/T) to +inf in fp32, so the reference ranking collapses to
"classes present among the top-200 (ascending) then absent (ascending)".
The device therefore only needs to FLAG, per (query, class-slot), whether
the slot could contain a top-200 member; the host recomputes flagged
slots exactly and replays the reference-equivalent accumulation.

Device strategy (shard train bank over N across 8 cores):
  - Host reorders x_train columns by class into a shared 4x2048-column
    grid per core (same slot structure on all cores; only the class
    assigned to each slot differs). Each slot occupies the same offset
    range in both halves of its group, so a single tensor-tensor max
    folds same-class column pairs.
  - Per core, per (group, query-block): 8 fp8 DoubleRow matmuls
    (contraction 256 each) -> PSUM [128, 2048] f32; the scalar engine
    casts the h1 half [0:1024] to SBUF bf16 (banks 1-2) while one raw
    InstTensorTensor max on the vector engine folds that bf16 tile
    against the h2 half read directly from PSUM f32 (banks 3-4) --
    one op per engine per iteration, so neither engine chain ever
    backpressures the PE's PSUM rotation. The folded tile is DMA'd to
    HBM.
  - Host decodes the folded bf16 sims, estimates the per-query top-200
    threshold, flags slots whose folded max is near/above it, recomputes
    those slots exactly in fp32, and runs the exact accumulation
    (fp32 exp -> scatter-add -> stable argsort), with per-query full
    fallback for ties/pathologies.
"""

import os
import sys

for _p in ("/opt/trn_rl_repo",):
    if _p not in sys.path and os.path.isdir(_p):
        sys.path.insert(0, _p)

import numpy as np

import concourse.mybir as mybir
import concourse.tile as tile
from concourse import bacc
from concourse.bass_utils import run_bass_kernel_spmd

# Problem shapes (hardcoded per spec)
B, N, D = 4096, 65536, 512
NUM_CLASSES = 100
KNN_K = 200
KNN_T = 0.1
NCORES = 8

KT = D // 128  # 4 contraction tiles
QB = B // 128  # 32 query blocks of 128
NG = 4  # PSUM groups per core
GW = 2048  # columns per group (4 PSUM banks)
HALF = GW // 2
CPC = NG * GW  # 8192 columns per core
FPC = NG * HALF  # 4096 folded columns per core
WARM_MMS = 24  # dummy matmuls issued at t=0 so HAM un-throttles the PE
# (and the DMA-wait head overlaps the 3.4us warmup window)
XCH = 8  # x DMA chunks (512 queries each)

SLACK = 5.6  # fp8 matmul noise (~4.5 sigma) + bf16 rounding
CUT = SLACK + 3.2  # slot-flag margin below the t0 threshold estimate
NEG = -1.0e30

_PROG = None
LAST_INFO = {}


def _tt_max(nc, out_ap, a_ap, b_ap):
    """Raw elementwise tensor-tensor max on the vector engine.

    bass has no public wrapper for InstTensorTensor, but it is the only
    DVE op with a 2x_1p uop (2 results/cycle on packed bf16) — the
    scalar_tensor_tensor wrapper runs at 1x.
    """
    ve = nc.vector
    return ve.add_instruction(
        mybir.InstTensorTensor(
            name=nc.get_next_instruction_name(),
            op=mybir.AluOpType.max,
            ins=[ve.lower_ap(a_ap), ve.lower_ap(b_ap)],
            outs=[ve.lower_ap(out_ap)],
        )
    )


def _build_program():
    nc = bacc.Bacc(
        "TRN2", target_bir_lowering=False, debug=False, num_devices=NCORES
    )
    f32 = mybir.dt.float32
    bf16 = mybir.dt.bfloat16
    f8 = mybir.dt.float8e4
    XW = B // XCH

    xT_d = nc.dram_tensor("xT", (D, B), f8, kind="ExternalInput").ap()
    wT_d = nc.dram_tensor("wT", (D, CPC), f8, kind="ExternalInput").ap()
    fold_d = nc.dram_tensor("fold", (B, FPC), bf16, kind="ExternalOutput").ap()

    from contextlib import ExitStack

    with tile.TileContext(nc) as tc:
        # PE warmup: dummy matmuls on garbage data, issued before any DMA
        # dependency, so the HAM clock-gate reaches 8/8 while the startup
        # DMAs are still in flight. Scoped pools so the PSUM bank is
        # returned before the main loop's double-buffered tiles claim
        # all 8 banks.
        with tc.tile_pool(name="warms", bufs=1) as wsp, tc.tile_pool(
            name="warmp", bufs=1, space="PSUM"
        ) as wpp:
            junk = wsp.tile([128, 2, 640], f8, tag="junk")
            nc.vector.memset(junk[:], 1.0)
            wps = wpp.tile([128, 512], f32, tag="wps")
            for r in range(WARM_MMS):
                nc.tensor.matmul(
                    wps[:],
                    junk[:, :, 0:128],
                    junk[:, :, 128:640],
                    start=True,
                    stop=True,
                    perf_mode=mybir.MatmulPerfMode.DoubleRow,
                )

        with ExitStack() as ctx:
            xpool = ctx.enter_context(tc.tile_pool(name="xp", bufs=1))
            wpool = ctx.enter_context(tc.tile_pool(name="wp", bufs=2))
            spool = ctx.enter_context(tc.tile_pool(name="sp", bufs=3))
            fpool = ctx.enter_context(tc.tile_pool(name="fp", bufs=3))
            ppool = ctx.enter_context(tc.tile_pool(name="pp", bufs=2, space="PSUM"))

            xsb = xpool.tile([128, KT, B], f8, tag="x")

            for g in range(NG):
                wt = wpool.tile([128, KT, GW], f8, tag="w")
                if g == 0:
                    # First-use-ordered startup: the first matmul
                    # (kp=0, t=0, block 0) needs x k0/k1 of query chunk 0
                    # plus w k0/k1 cols [0:512] -- ~256 KB -- then the
                    # rest streams in first-use order.
                    for kp in range(KT // 2):
                        for k in (2 * kp, 2 * kp + 1):
                            nc.sync.dma_start(
                                xsb[:, k, 0:XW],
                                xT_d[k * 128 : (k + 1) * 128, 0:XW],
                            )
                        for t in range(GW // 512):
                            for k in (2 * kp, 2 * kp + 1):
                                nc.sync.dma_start(
                                    wt[:, k, t * 512 : (t + 1) * 512],
                                    wT_d[
                                        k * 128 : (k + 1) * 128,
                                        t * 512 : (t + 1) * 512,
                                    ],
                                )
                    for c in range(1, XCH):
                        for k in range(KT):
                            nc.sync.dma_start(
                                xsb[:, k, c * XW : (c + 1) * XW],
                                xT_d[k * 128 : (k + 1) * 128, c * XW : (c + 1) * XW],
                            )
                else:
                    for k in range(KT):
                        nc.sync.dma_start(
                            wt[:, k, :],
                            wT_d[k * 128 : (k + 1) * 128, g * GW : (g + 1) * GW],
                        )
                for b in range(QB):
                    ps = ppool.tile([128, GW], f32, tag="ps")
                    for kp in range(KT // 2):
                        for t in range(GW // 512):
                            nc.tensor.matmul(
                                ps[:, t * 512 : (t + 1) * 512],
                                xsb[:, 2 * kp : 2 * kp + 2, b * 128 : (b + 1) * 128],
                                wt[:, 2 * kp : 2 * kp + 2, t * 512 : (t + 1) * 512],
                                start=(kp == 0),
                                stop=(kp == KT // 2 - 1),
                                perf_mode=mybir.MatmulPerfMode.DoubleRow,
                            )
                    # Two copy+fold pairs of 512 cols each: PSUM banks are
                    # released incrementally (bank1 by copy1, bank3 by
                    # fold1, ...), keeping every link of the chain shorter
                    # than the PE's 8-matmul production period so the
                    # 2-deep PSUM rotation never backpressures the PE.
                    sbf = spool.tile([128, HALF], bf16, tag="s")
                    f1 = fpool.tile([128, HALF], bf16, tag="f")
                    nc.scalar.copy(sbf[:, 0:512], ps[:, 0:512])
                    _tt_max(
                        nc, f1[:, 0:512], sbf[:, 0:512], ps[:, HALF : HALF + 512]
                    )
                    nc.scalar.copy(sbf[:, 512:HALF], ps[:, 512:HALF])
                    _tt_max(
                        nc, f1[:, 512:HALF], sbf[:, 512:HALF], ps[:, HALF + 512 : GW]
                    )
                    nc.sync.dma_start(
                        fold_d[
                            b * 128 : (b + 1) * 128,
                            g * HALF : (g + 1) * HALF,
                        ],
                        f1[:],
                    )

    nc.compile()
    return nc


def _get_program():
    global _PROG
    if _PROG is None:
        _PROG = _build_program()
    return _PROG


def _plan_layout(y_train):
    """Class-pure slot layout on a shared 4x2048 grid, identical across cores.

    Each slot j has a fixed half-width h[j] and (group, offset) shared by
    all cores; core i places its j-th largest class there, split into two
    h[j]-column halves at the same offset in each half of the group (so
    the device fold pairs same-class columns). Rows that do not fit are
    computed exactly on the host.

    Returns (colmap, host_rows):
      colmap: int64 [NCORES, CPC] -> original x_train row, -1 pad
      host_rows: int64 [H] train rows computed exactly on the host
    """
    cnt = np.bincount(y_train, minlength=NUM_CLASSES)
    by_class = np.argsort(y_train, kind="stable")
    starts = np.zeros(NUM_CLASSES + 1, dtype=np.int64)
    np.cumsum(cnt, out=starts[1:])

    cls_desc = np.argsort(-cnt, kind="stable")
    core_classes = [[] for _ in range(NCORES)]
    for idx, c in enumerate(cls_desc):
        r, i = divmod(idx, NCORES)
        if r % 2:
            i = NCORES - 1 - i
        core_classes[i].append(int(c))
    S = max(len(cc) for cc in core_classes)
    for cc in core_classes:
        while len(cc) < S:
            cc.append(-1)

    # Shared half-widths: the widest class assigned to each slot position.
    h = np.zeros(S, dtype=np.int64)
    for j in range(S):
        h[j] = max(
            (cnt[cc[j]] + 1) // 2 if cc[j] >= 0 else 0 for cc in core_classes
        )
        h[j] = max(h[j], 1)
    # Fit the grid: total half-capacity is NG * HALF.
    cap = NG * HALF
    while h.sum() > cap:  # shed excess one column at a time, widest first
        j = int(np.argmax(h))
        h[j] -= 1

    # Pack slots into the 4 group bins (first-fit decreasing).
    order = sorted(range(S), key=lambda j: -h[j])
    bin_used = [0] * NG
    slot_group = np.zeros(S, dtype=np.int64)
    slot_off = np.zeros(S, dtype=np.int64)
    for j in order:
        g = min(
            (i for i in range(NG) if bin_used[i] + h[j] <= HALF),
            key=lambda i: HALF - bin_used[i],
            default=None,
        )
        if g is None:
            # Shrink to the largest remaining bin.
            g = int(np.argmax([HALF - u for u in bin_used]))
            h[j] = HALF - bin_used[g]
        slot_group[j] = g
        slot_off[j] = bin_used[g]
        bin_used[g] += h[j]

    colmap = np.full((NCORES, CPC), -1, dtype=np.int64)
    host_rows = []
    for j in range(S):
        g, off, hj = int(slot_group[j]), int(slot_off[j]), int(h[j])
        for i in range(NCORES):
            c = core_classes[i][j]
            if c < 0:
                continue
            n = int(cnt[c])
            rows = by_class[starts[c] : starts[c] + n]
            keep = min(n, 2 * hj)
            n1 = min(hj, keep)
            c1 = g * GW + off
            c2 = g * GW + HALF + off
            colmap[i, c1 : c1 + n1] = rows[0:n1]
            if keep > n1:
                colmap[i, c2 : c2 + keep - n1] = rows[n1:keep]
            if n > keep:
                host_rows.extend(rows[keep:n])

    host_rows = np.array(sorted(host_rows), dtype=np.int64)
    return colmap, host_rows


def _host_merge(x, x_train, y_train, folded, colmap, host_rows):
    """Exact top-200 presence -> class scores -> ranking.

    folded: f32 [NCORES, B, FPC] pair-maxes of fp8 sims (bf16 rounded).
    Values serve only to locate candidates: every folded PAIR that could
    contain a top-200 member gets both of its columns recomputed exactly
    (~300 pairs/query), so no device value is ever used as a final
    candidate. Host FLOPs stay ~3 GFLOP (single-CPU container).
    """
    H = host_rows.shape[0]
    if H:
        hostV = (x @ x_train[host_rows].T).astype(np.float64)  # [B, H]
        host_class = y_train[host_rows]
    else:
        hostV = np.zeros((B, 0))
        host_class = np.zeros(0, dtype=y_train.dtype)

    # t0: estimate of the top-200 threshold from folded values + hostV.
    flat = np.ascontiguousarray(folded.transpose(1, 0, 2)).reshape(
        B, NCORES * FPC
    )
    A0 = np.concatenate([flat, hostV.astype(np.float32)], axis=1)
    kth = A0.shape[1] - KNN_K
    t0 = np.partition(A0, kth, axis=1)[:, kth].astype(np.float64)
    del A0

    # Candidate pairs: folded value near/above t0.
    mask = flat >= (t0[:, None].astype(np.float32) - CUT)
    qidx, fcol = np.nonzero(mask)
    del mask
    LAST_INFO["cand_pairs"] = int(qidx.size)
    core = fcol // FPC
    off = fcol % FPC
    g = off // HALF
    o = off % HALF
    c1 = g * GW + o
    c2 = g * GW + HALF + o
    r1 = colmap[core, c1]
    r2 = colmap[core, c2]
    rows2 = np.stack([r1, r2], axis=1).ravel()
    q2 = np.repeat(qidx, 2)
    valid = rows2 >= 0
    rows2 = rows2[valid]
    q2 = q2[valid]

    # Exact sims for candidate columns (chunked row-wise dot products).
    vals = np.empty(rows2.size, dtype=np.float64)
    CH = 65536
    for s in range(0, rows2.size, CH):
        e = min(s + CH, rows2.size)
        vals[s:e] = np.einsum(
            "nd,nd->n", x_train[rows2[s:e]], x[q2[s:e]], optimize=True
        )
    labels2 = y_train[rows2]

    # Per-query candidate table [B, MAXC] (+ hostV appended).
    counts = np.bincount(q2, minlength=B)
    MAXC = int(counts.max())
    starts = np.zeros(B + 1, dtype=np.int64)
    np.cumsum(counts, out=starts[1:])
    # q2 is already sorted (nonzero row-major); position within query:
    pos = np.arange(q2.size) - starts[q2]
    Acand = np.full((B, MAXC), NEG, dtype=np.float64)
    Lcand = np.zeros((B, MAXC), dtype=np.int64)
    Acand[q2, pos] = vals
    Lcand[q2, pos] = labels2

    A = np.concatenate([Acand, hostV], axis=1)
    L = np.concatenate(
        [Lcand, np.broadcast_to(host_class, (B, H))], axis=1
    )
    kth = A.shape[1] - KNN_K
    t1 = np.partition(A, kth, axis=1)[:, kth]
    sel = np.argpartition(-A, KNN_K - 1, axis=1)[:, :KNN_K]
    rowix = np.arange(B)[:, None]
    sel_v = A[rowix, sel]

    full_fallback = set()
    # Boundary ties -> per-query fallback (argpartition splits arbitrarily)
    vmin = sel_v.min(axis=1)
    tie = (A == vmin[:, None]).sum(axis=1) != (sel_v == vmin[:, None]).sum(axis=1)
    for q in np.nonzero(tie)[0]:
        full_fallback.add(int(q))
    # Guards: flag logic assumes t1 close to t0, and positive thresholds.
    for q in np.nonzero((t1 < 1.0) | (t1 < t0 - CUT + SLACK + 0.5))[0]:
        full_fallback.add(int(q))
    LAST_INFO["fallback_rows"] = len(full_fallback)

    labels = L[rowix, sel]  # [B, K]

    scores = np.zeros((B, NUM_CLASSES), dtype=np.float32)
    with np.errstate(over="ignore"):
        wts = np.exp(sel_v.astype(np.float32) / np.float32(KNN_T))
    ok = np.ones(B, dtype=bool)
    for q in full_fallback:
        ok[q] = False
    qs = np.nonzero(ok)[0]
    np.add.at(
        scores,
        (np.repeat(qs, KNN_K), labels[qs].ravel()),
        wts[qs].ravel(),
    )

    if full_fallback:
        qfb = np.array(sorted(full_fallback))
        sims_fb = x[qfb].astype(np.float64) @ x_train.T.astype(np.float64)
        for i, q in enumerate(qfb):
            sims = sims_fb[i]
            cand = np.argpartition(-sims, KNN_K + 56)[: KNN_K + 56]
            order = cand[np.lexsort((cand, -sims[cand]))][:KNN_K]
            lab = y_train[order]
            with np.errstate(over="ignore"):
                wq = np.exp(sims[order].astype(np.float32) / np.float32(KNN_T))
            scores[q] = 0.0
            np.add.at(scores[q], lab, wq)

    return np.argsort(-scores, axis=1, kind="stable").astype(np.int32)


def kernel(x, x_train, y_train):
    x = np.asarray(x, dtype=np.float32)
    x_train = np.asarray(x_train, dtype=np.float32)
    y_train = np.asarray(y_train).astype(np.int64)

    colmap, host_rows = _plan_layout(y_train)
    nc = _get_program()

    f8np = mybir.dt.np(mybir.dt.float8e4)
    xtr_T = np.ascontiguousarray(x_train.T)  # [D, N] f32
    xT = np.ascontiguousarray(x.T).astype(f8np)

    in_maps = []
    for i in range(NCORES):
        wTi = np.zeros((D, CPC), dtype=f8np)
        real = colmap[i] >= 0
        wTi[:, real] = xtr_T[:, colmap[i, real]].astype(f8np)
        in_maps.append({"xT": xT, "wT": wTi})

    res = run_bass_kernel_spmd(nc, in_maps, core_ids=list(range(NCORES)))
    LAST_INFO["exec_time_ns"] = res.exec_time_ns
    LAST_INFO["results"] = res

    folded = np.stack(
        [
            np.asarray(res.results[c]["fold"]).astype(np.float32)
            for c in range(NCORES)
        ]
    )  # [NCORES, B, FPC] f32
    return _host_merge(x, x_train, y_train, folded, colmap, host_rows)


# revision 19
# speedup vs baseline: 1.0946x; 1.0062x over previous
"""KNN classifier kernel for Trainium2 (8 NeuronCores, Bass/Tile).

Problem (nn_KNNClassifier): given queries x [4096, 512], train bank
x_train [65536, 512], labels y_train [65536] (100 classes), compute for
each query the top-200 neighbors by dot-product similarity, weight them
by exp(sim/0.1), accumulate per-class scores, and return the descending
argsort of class scores -> int32 [4096, 100].

Key structural fact: with T=0.1 every top-200 sim (~70+) overflows
exp(sim/T) to +inf in fp32, so the reference ranking collapses to
"classes present among the top-200 (ascending) then absent (ascending)".
The device therefore only needs to FLAG, per (query, class-slot), whether
the slot could contain a top-200 member; the host recomputes flagged
slots exactly and replays the reference-equivalent accumulation.

Device strategy (shard train bank over N across 8 cores):
  - Host reorders x_train columns by class into a shared 4x2048-column
    grid per core (same slot structure on all cores; only the class
    assigned to each slot differs). Each slot occupies the same offset
    range in both halves of its group, so a single tensor-tensor max
    folds same-class column pairs.
  - Per core, per (group, query-block): 8 fp8 DoubleRow matmuls
    (contraction 256 each) -> PSUM [128, 2048] f32; the scalar engine
    casts the h1 half [0:1024] to SBUF bf16 (banks 1-2) while one raw
    InstTensorTensor max on the vector engine folds that bf16 tile
    against the h2 half read directly from PSUM f32 (banks 3-4) --
    one op per engine per iteration, so neither engine chain ever
    backpressures the PE's PSUM rotation. The folded tile is DMA'd to
    HBM.
  - Host decodes the folded bf16 sims, estimates the per-query top-200
    threshold, flags slots whose folded max is near/above it, recomputes
    those slots exactly in fp32, and runs the exact accumulation
    (fp32 exp -> scatter-add -> stable argsort), with per-query full
    fallback for ties/pathologies.
"""

import os
import sys

for _p in ("/opt/trn_rl_repo",):
    if _p not in sys.path and os.path.isdir(_p):
        sys.path.insert(0, _p)

import numpy as np

import concourse.mybir as mybir
import concourse.tile as tile
from concourse import bacc
from concourse.bass_utils import run_bass_kernel_spmd

# Problem shapes (hardcoded per spec)
B, N, D = 4096, 65536, 512
NUM_CLASSES = 100
KNN_K = 200
KNN_T = 0.1
NCORES = 8

KT = D // 128  # 4 contraction tiles
QB = B // 128  # 32 query blocks of 128
NG = 4  # PSUM groups per core
GW = 2048  # columns per group (4 PSUM banks)
HALF = GW // 2
CPC = NG * GW  # 8192 columns per core
FPC = NG * HALF  # 4096 folded columns per core
WARM_MMS = 24  # dummy matmuls issued at t=0 so HAM un-throttles the PE
# (and the DMA-wait head overlaps the 3.4us warmup window)
XCH = 8  # x DMA chunks (512 queries each)

SLACK = 5.6  # fp8 matmul noise (~4.5 sigma) + bf16 rounding
CUT = SLACK + 3.2  # slot-flag margin below the t0 threshold estimate
NEG = -1.0e30

_PROG = None
LAST_INFO = {}


def _tt_max(nc, out_ap, a_ap, b_ap):
    """Raw elementwise tensor-tensor max on the vector engine.

    bass has no public wrapper for InstTensorTensor, but it is the only
    DVE op with a 2x_1p uop (2 results/cycle on packed bf16) — the
    scalar_tensor_tensor wrapper runs at 1x.
    """
    ve = nc.vector
    return ve.add_instruction(
        mybir.InstTensorTensor(
            name=nc.get_next_instruction_name(),
            op=mybir.AluOpType.max,
            ins=[ve.lower_ap(a_ap), ve.lower_ap(b_ap)],
            outs=[ve.lower_ap(out_ap)],
        )
    )


def _build_program():
    nc = bacc.Bacc(
        "TRN2", target_bir_lowering=False, debug=False, num_devices=NCORES
    )
    f32 = mybir.dt.float32
    bf16 = mybir.dt.bfloat16
    f8 = mybir.dt.float8e4
    XW = B // XCH

    xT_d = nc.dram_tensor("xT", (D, B), f8, kind="ExternalInput").ap()
    wT_d = nc.dram_tensor("wT", (D, CPC), f8, kind="ExternalInput").ap()
    # Group-major fold layout: each (group, block) DMA writes one fully
    # contiguous 256 KB region instead of 128 strided 2 KB rows.
    fold_d = nc.dram_tensor(
        "fold", (NG, B, HALF), bf16, kind="ExternalOutput"
    ).ap()

    from contextlib import ExitStack

    with tile.TileContext(nc) as tc:
        # PE warmup: dummy matmuls on garbage data, issued before any DMA
        # dependency, so the HAM clock-gate reaches 8/8 while the startup
        # DMAs are still in flight. Scoped pools so the PSUM bank is
        # returned before the main loop's double-buffered tiles claim
        # all 8 banks.
        with tc.tile_pool(name="warms", bufs=1) as wsp, tc.tile_pool(
            name="warmp", bufs=1, space="PSUM"
        ) as wpp:
            junk = wsp.tile([128, 2, 640], f8, tag="junk")
            nc.vector.memset(junk[:], 1.0)
            wps = wpp.tile([128, 512], f32, tag="wps")
            for r in range(WARM_MMS):
                nc.tensor.matmul(
                    wps[:],
                    junk[:, :, 0:128],
                    junk[:, :, 128:640],
                    start=True,
                    stop=True,
                    perf_mode=mybir.MatmulPerfMode.DoubleRow,
                )

        with ExitStack() as ctx:
            xpool = ctx.enter_context(tc.tile_pool(name="xp", bufs=1))
            wpool = ctx.enter_context(tc.tile_pool(name="wp", bufs=2))
            spool = ctx.enter_context(tc.tile_pool(name="sp", bufs=3))
            fpool = ctx.enter_context(tc.tile_pool(name="fp", bufs=3))
            ppool = ctx.enter_context(tc.tile_pool(name="pp", bufs=2, space="PSUM"))

            xsb = xpool.tile([128, KT, B], f8, tag="x")

            for g in range(NG):
                wt = wpool.tile([128, KT, GW], f8, tag="w")
                if g == 0:
                    # First-use-ordered startup: the first matmul
                    # (kp=0, t=0, block 0) needs x k0/k1 of query chunk 0
                    # plus w k0/k1 cols [0:512] -- ~256 KB -- then the
                    # rest streams in first-use order.
                    for kp in range(KT // 2):
                        for k in (2 * kp, 2 * kp + 1):
                            nc.sync.dma_start(
                                xsb[:, k, 0:XW],
                                xT_d[k * 128 : (k + 1) * 128, 0:XW],
                            )
                        for t in range(GW // 512):
                            for k in (2 * kp, 2 * kp + 1):
                                nc.sync.dma_start(
                                    wt[:, k, t * 512 : (t + 1) * 512],
                                    wT_d[
                                        k * 128 : (k + 1) * 128,
                                        t * 512 : (t + 1) * 512,
                                    ],
                                )
                    for c in range(1, XCH):
                        for k in range(KT):
                            nc.sync.dma_start(
                                xsb[:, k, c * XW : (c + 1) * XW],
                                xT_d[k * 128 : (k + 1) * 128, c * XW : (c + 1) * XW],
                            )
                else:
                    for k in range(KT):
                        nc.sync.dma_start(
                            wt[:, k, :],
                            wT_d[k * 128 : (k + 1) * 128, g * GW : (g + 1) * GW],
                        )
                for b in range(QB):
                    ps = ppool.tile([128, GW], f32, tag="ps")
                    for kp in range(KT // 2):
                        for t in range(GW // 512):
                            nc.tensor.matmul(
                                ps[:, t * 512 : (t + 1) * 512],
                                xsb[:, 2 * kp : 2 * kp + 2, b * 128 : (b + 1) * 128],
                                wt[:, 2 * kp : 2 * kp + 2, t * 512 : (t + 1) * 512],
                                start=(kp == 0),
                                stop=(kp == KT // 2 - 1),
                                perf_mode=mybir.MatmulPerfMode.DoubleRow,
                            )
                    # Two copy+fold pairs of 512 cols each, on SEPARATE
                    # tiles (tile-granular dependency tracking would
                    # otherwise serialize copy2 behind fold1's read):
                    # PSUM banks release incrementally, so the 2-deep
                    # PSUM rotation never backpressures the PE.
                    sbfA = spool.tile([128, 512], bf16, tag="sA")
                    sbfB = spool.tile([128, 512], bf16, tag="sB")
                    f1 = fpool.tile([128, HALF], bf16, tag="f")
                    nc.scalar.copy(sbfA[:], ps[:, 0:512])
                    _tt_max(
                        nc, f1[:, 0:512], sbfA[:], ps[:, HALF : HALF + 512]
                    )
                    nc.scalar.copy(sbfB[:], ps[:, 512:HALF])
                    _tt_max(
                        nc, f1[:, 512:HALF], sbfB[:], ps[:, HALF + 512 : GW]
                    )
                    nc.sync.dma_start(
                        fold_d[g, b * 128 : (b + 1) * 128, :],
                        f1[:],
                    )

    nc.compile()
    return nc


def _get_program():
    global _PROG
    if _PROG is None:
        _PROG = _build_program()
    return _PROG


def _plan_layout(y_train):
    """Class-pure slot layout on a shared 4x2048 grid, identical across cores.

    Each slot j has a fixed half-width h[j] and (group, offset) shared by
    all cores; core i places its j-th largest class there, split into two
    h[j]-column halves at the same offset in each half of the group (so
    the device fold pairs same-class columns). Rows that do not fit are
    computed exactly on the host.

    Returns (colmap, host_rows):
      colmap: int64 [NCORES, CPC] -> original x_train row, -1 pad
      host_rows: int64 [H] train rows computed exactly on the host
    """
    cnt = np.bincount(y_train, minlength=NUM_CLASSES)
    by_class = np.argsort(y_train, kind="stable")
    starts = np.zeros(NUM_CLASSES + 1, dtype=np.int64)
    np.cumsum(cnt, out=starts[1:])

    cls_desc = np.argsort(-cnt, kind="stable")
    core_classes = [[] for _ in range(NCORES)]
    for idx, c in enumerate(cls_desc):
        r, i = divmod(idx, NCORES)
        if r % 2:
            i = NCORES - 1 - i
        core_classes[i].append(int(c))
    S = max(len(cc) for cc in core_classes)
    for cc in core_classes:
        while len(cc) < S:
            cc.append(-1)

    # Shared half-widths: the widest class assigned to each slot position.
    h = np.zeros(S, dtype=np.int64)
    for j in range(S):
        h[j] = max(
            (cnt[cc[j]] + 1) // 2 if cc[j] >= 0 else 0 for cc in core_classes
        )
        h[j] = max(h[j], 1)
    # Fit the grid: total half-capacity is NG * HALF.
    cap = NG * HALF
    while h.sum() > cap:  # shed excess one column at a time, widest first
        j = int(np.argmax(h))
        h[j] -= 1

    # Pack slots into the 4 group bins (first-fit decreasing).
    order = sorted(range(S), key=lambda j: -h[j])
    bin_used = [0] * NG
    slot_group = np.zeros(S, dtype=np.int64)
    slot_off = np.zeros(S, dtype=np.int64)
    for j in order:
        g = min(
            (i for i in range(NG) if bin_used[i] + h[j] <= HALF),
            key=lambda i: HALF - bin_used[i],
            default=None,
        )
        if g is None:
            # Shrink to the largest remaining bin.
            g = int(np.argmax([HALF - u for u in bin_used]))
            h[j] = HALF - bin_used[g]
        slot_group[j] = g
        slot_off[j] = bin_used[g]
        bin_used[g] += h[j]

    colmap = np.full((NCORES, CPC), -1, dtype=np.int64)
    host_rows = []
    for j in range(S):
        g, off, hj = int(slot_group[j]), int(slot_off[j]), int(h[j])
        for i in range(NCORES):
            c = core_classes[i][j]
            if c < 0:
                continue
            n = int(cnt[c])
            rows = by_class[starts[c] : starts[c] + n]
            keep = min(n, 2 * hj)
            n1 = min(hj, keep)
            c1 = g * GW + off
            c2 = g * GW + HALF + off
            colmap[i, c1 : c1 + n1] = rows[0:n1]
            if keep > n1:
                colmap[i, c2 : c2 + keep - n1] = rows[n1:keep]
            if n > keep:
                host_rows.extend(rows[keep:n])

    host_rows = np.array(sorted(host_rows), dtype=np.int64)
    return colmap, host_rows


def _host_merge(x, x_train, y_train, folded, colmap, host_rows):
    """Exact top-200 presence -> class scores -> ranking.

    folded: f32 [NCORES, B, FPC] pair-maxes of fp8 sims (bf16 rounded).
    Values serve only to locate candidates: every folded PAIR that could
    contain a top-200 member gets both of its columns recomputed exactly
    (~300 pairs/query), so no device value is ever used as a final
    candidate. Host FLOPs stay ~3 GFLOP (single-CPU container).
    """
    H = host_rows.shape[0]
    if H:
        hostV = (x @ x_train[host_rows].T).astype(np.float64)  # [B, H]
        host_class = y_train[host_rows]
    else:
        hostV = np.zeros((B, 0))
        host_class = np.zeros(0, dtype=y_train.dtype)

    # t0: estimate of the top-200 threshold from folded values + hostV.
    flat = np.ascontiguousarray(folded.transpose(1, 0, 2)).reshape(
        B, NCORES * FPC
    )
    A0 = np.concatenate([flat, hostV.astype(np.float32)], axis=1)
    kth = A0.shape[1] - KNN_K
    t0 = np.partition(A0, kth, axis=1)[:, kth].astype(np.float64)
    del A0

    # Candidate pairs: folded value near/above t0.
    mask = flat >= (t0[:, None].astype(np.float32) - CUT)
    qidx, fcol = np.nonzero(mask)
    del mask
    LAST_INFO["cand_pairs"] = int(qidx.size)
    core = fcol // FPC
    off = fcol % FPC
    g = off // HALF
    o = off % HALF
    c1 = g * GW + o
    c2 = g * GW + HALF + o
    r1 = colmap[core, c1]
    r2 = colmap[core, c2]
    rows2 = np.stack([r1, r2], axis=1).ravel()
    q2 = np.repeat(qidx, 2)
    valid = rows2 >= 0
    rows2 = rows2[valid]
    q2 = q2[valid]

    # Exact sims for candidate columns (chunked row-wise dot products).
    vals = np.empty(rows2.size, dtype=np.float64)
    CH = 65536
    for s in range(0, rows2.size, CH):
        e = min(s + CH, rows2.size)
        vals[s:e] = np.einsum(
            "nd,nd->n", x_train[rows2[s:e]], x[q2[s:e]], optimize=True
        )
    labels2 = y_train[rows2]

    # Per-query candidate table [B, MAXC] (+ hostV appended).
    counts = np.bincount(q2, minlength=B)
    MAXC = int(counts.max())
    starts = np.zeros(B + 1, dtype=np.int64)
    np.cumsum(counts, out=starts[1:])
    # q2 is already sorted (nonzero row-major); position within query:
    pos = np.arange(q2.size) - starts[q2]
    Acand = np.full((B, MAXC), NEG, dtype=np.float64)
    Lcand = np.zeros((B, MAXC), dtype=np.int64)
    Acand[q2, pos] = vals
    Lcand[q2, pos] = labels2

    A = np.concatenate([Acand, hostV], axis=1)
    L = np.concatenate(
        [Lcand, np.broadcast_to(host_class, (B, H))], axis=1
    )
    kth = A.shape[1] - KNN_K
    t1 = np.partition(A, kth, axis=1)[:, kth]
    sel = np.argpartition(-A, KNN_K - 1, axis=1)[:, :KNN_K]
    rowix = np.arange(B)[:, None]
    sel_v = A[rowix, sel]

    full_fallback = set()
    # Boundary ties -> per-query fallback (argpartition splits arbitrarily)
    vmin = sel_v.min(axis=1)
    tie = (A == vmin[:, None]).sum(axis=1) != (sel_v == vmin[:, None]).sum(axis=1)
    for q in np.nonzero(tie)[0]:
        full_fallback.add(int(q))
    # Guards: flag logic assumes t1 close to t0, and positive thresholds.
    for q in np.nonzero((t1 < 1.0) | (t1 < t0 - CUT + SLACK + 0.5))[0]:
        full_fallback.add(int(q))
    LAST_INFO["fallback_rows"] = len(full_fallback)

    labels = L[rowix, sel]  # [B, K]

    scores = np.zeros((B, NUM_CLASSES), dtype=np.float32)
    with np.errstate(over="ignore"):
        wts = np.exp(sel_v.astype(np.float32) / np.float32(KNN_T))
    ok = np.ones(B, dtype=bool)
    for q in full_fallback:
        ok[q] = False
    qs = np.nonzero(ok)[0]
    np.add.at(
        scores,
        (np.repeat(qs, KNN_K), labels[qs].ravel()),
        wts[qs].ravel(),
    )

    if full_fallback:
        qfb = np.array(sorted(full_fallback))
        sims_fb = x[qfb].astype(np.float64) @ x_train.T.astype(np.float64)
        for i, q in enumerate(qfb):
            sims = sims_fb[i]
            cand = np.argpartition(-sims, KNN_K + 56)[: KNN_K + 56]
            order = cand[np.lexsort((cand, -sims[cand]))][:KNN_K]
            lab = y_train[order]
            with np.errstate(over="ignore"):
                wq = np.exp(sims[order].astype(np.float32) / np.float32(KNN_T))
            scores[q] = 0.0
            np.add.at(scores[q], lab, wq)

    return np.argsort(-scores, axis=1, kind="stable").astype(np.int32)


def kernel(x, x_train, y_train):
    x = np.asarray(x, dtype=np.float32)
    x_train = np.asarray(x_train, dtype=np.float32)
    y_train = np.asarray(y_train).astype(np.int64)

    colmap, host_rows = _plan_layout(y_train)
    nc = _get_program()

    f8np = mybir.dt.np(mybir.dt.float8e4)
    xtr_T = np.ascontiguousarray(x_train.T)  # [D, N] f32
    xT = np.ascontiguousarray(x.T).astype(f8np)

    in_maps = []
    for i in range(NCORES):
        wTi = np.zeros((D, CPC), dtype=f8np)
        real = colmap[i] >= 0
        wTi[:, real] = xtr_T[:, colmap[i, real]].astype(f8np)
        in_maps.append({"xT": xT, "wT": wTi})

    res = run_bass_kernel_spmd(nc, in_maps, core_ids=list(range(NCORES)))
    LAST_INFO["exec_time_ns"] = res.exec_time_ns
    LAST_INFO["results"] = res

    folded = np.stack(
        [
            np.asarray(res.results[c]["fold"])
            .transpose(1, 0, 2)  # [NG, B, HALF] -> [B, NG, HALF]
            .reshape(B, FPC)
            .astype(np.float32)
            for c in range(NCORES)
        ]
    )  # [NCORES, B, FPC] f32, fold col = g * HALF + o
    return _host_merge(x, x_train, y_train, folded, colmap, host_rows)
